# revision 53
# baseline (speedup 1.0000x reference)
"""GCN pipeline (proj + 2x GCNConv + GraphNorm + spot-softmax aggregation +
MLP head) on 8 trn2 NeuronCores via Bass/Tile.

Sharding: core c owns nodes [c*NSH,(c+1)*NSH) and spots [c*SSH,(c+1)*SSH).
Activations are feature-major [H, NSH] in SBUF.

GCN layers: per-layer bf16 node-major table t' = dinv*(h@W) (128 bf16/row =
256B) is AllGathered; dst-sorted edges gather rows by src (dma_gather, 4 SWDGE
queues) and are scatter-accumulated per 128-dst tile with one-hot matmuls on
the PE (stationary = gathered rows, moving = one-hot, psum accumulates
feature-major [H,128] added straight into agg).

Spot aggregation: softmax without max-subtraction (scores are O(1)); each core
builds local rows [e*h | e] bf16 (e = exp(score)), gathers them in
spot-sorted order from local HBM, PE-scatters into per-128-spot-group partial
sums [S,97] f32, then one ReduceScatter(add) gives each core its own spot
shard; normalize by the e-sum column and run the MLP head locally.
"""
import sys, os
sys.path.insert(0, '/opt/trn_rl_repo')
import numpy as np

N_CORES = 8


class Cfg:
    def __init__(self, n_nodes=50000, n_edges=800000, in_dim=128, hid=96,
                 attn_hid=32, out_dim=16, n_spots=5000, eps=1e-5):
        assert n_nodes % N_CORES == 0 and n_spots % N_CORES == 0
        self.N, self.E, self.IN, self.H = n_nodes, n_edges, in_dim, hid
        self.AH, self.OD, self.S, self.EPS = attn_hid, out_dim, n_spots, eps
        self.NSH = n_nodes // N_CORES
        self.SSH = n_spots // N_CORES
        self.NT = (self.NSH + 127) // 128          # node tiles per core
        self.NTP = self.NT * 128                   # padded rows per core
        self.SPLT = self.NT // 2                   # table piece-0 tiles
        self.SPL = self.SPLT * 128                 # table split row
        self.QR = [self.SPL, self.NTP - self.SPL]  # AG chunk rows per core
        self.CHR = [N_CORES * q for q in self.QR]  # chunk table rows
        assert max(self.CHR) < (1 << 15)
        self.ST = (self.SSH + 127) // 128          # spot groups per core
        self.GT = (n_spots + 127) // 128           # global spot groups
        self.TROW = 128                            # table row elems (bf16)
        self.CALL = 2560                           # gather slots per call


def _wrap_idx(flat):
    """int16 slot list -> [128, n/16] wrapped layout (replicated 8x)."""
    n = len(flat)
    assert n % 16 == 0
    w = flat.reshape(n // 16, 16).T.astype(np.int16)   # [16, n/16]
    return np.tile(w, (8, 1))


def _prep_edges(cfg, src, dst):
    """Per-core dst-sorted edge structure, uniform across cores for SPMD.

    Phase h covers edges whose src local row is in AG chunk h (local rows
    [h*QROWS,(h+1)*QROWS)); chunk-h table = concat over cores of that row
    range, so phase-0 gathers can start when the first chunk AllGather lands.
    Within a phase: tiles 0..NT-1, S[t,h] blocks each. Returns per-core
    wrapped idx arrays, per-block dst-local columns, and shared meta.
    """
    NSH, NT = cfg.NSH, cfg.NT
    Q0, Q1 = cfg.QR
    src_core = src // NSH
    src_r = src % NSH                                  # local row (= node id)
    core_of = dst // NSH
    per_core = []
    cnt_all = np.zeros((N_CORES, NT, 2), np.int64)
    for c in range(N_CORES):
        m = core_of == c
        h_c = (src_r[m] >= Q0).astype(np.int64)
        r_c = np.where(h_c == 1,
                       src_core[m] * Q1 + src_r[m] - Q0,
                       src_core[m] * Q0 + src_r[m])    # row within chunk
        d_c = dst[m] - c * NSH
        t_c = d_c // 128
        key = t_c * 2 + h_c
        order = np.argsort(key, kind='stable')
        per_core.append((r_c[order], d_c[order], key[order]))
        cnt_all[c] = np.bincount(key, minlength=NT * 2).reshape(NT, 2)
    S = (cnt_all.max(axis=0) + 127) // 128             # [NT, 2] blocks
    assert S.min() >= 1      # every tile closes in phase 1 (finalization)
    nblk = [int(S[:, h].sum()) for h in (0, 1)]
    slots = [n * 128 for n in nblk]
    b0 = np.zeros((NT, 2), np.int64)
    for h in (0, 1):
        b0[:, h] = np.cumsum(S[:, h]) - S[:, h]
    idx_w, dl_w = [], []
    for c in range(N_CORES):
        r_c, d_c, key = per_core[c]
        bounds = np.searchsorted(key, np.arange(NT * 2 + 1))
        idx_flat = np.zeros(slots[0] + slots[1], np.int64)
        dl_flat = -np.ones(slots[0] + slots[1], np.float32)
        for t in range(NT):
            for h in (0, 1):
                lo, hi = bounds[t * 2 + h], bounds[t * 2 + h + 1]
                n = hi - lo
                off = (0 if h == 0 else slots[0]) + int(b0[t, h]) * 128
                idx_flat[off:off + n] = r_c[lo:hi]
                dl_flat[off:off + n] = (d_c[lo:hi] % 128).astype(np.float32)
        idx_w.append(_wrap_idx(idx_flat))
        dl_w.append(np.ascontiguousarray(
            dl_flat.reshape(-1, 128).T))               # [128, nblk_tot]
    meta = dict(S=S, b0=b0, nblk=nblk, slots=slots)
    return idx_w, dl_w, meta


def _prep_spots(cfg, cts):
    """Per-core spot-sorted local-node structure for the partial-sum scatter.

    Local nodes sorted by global spot id, split into 128-slot blocks aligned
    to 128-spot groups (group g covers spots [g*128,(g+1)*128)). Uniform
    block counts (max over cores) for SPMD.
    """
    NSH, GT, SPL = cfg.NSH, cfg.GT, cfg.SPL
    nodes = np.arange(NSH)
    per_core = []
    cnt_all = np.zeros((N_CORES, GT, 2), np.int64)
    for c in range(N_CORES):
        p = cts[c * NSH:(c + 1) * NSH]
        hh = (nodes >= SPL).astype(np.int64)
        key = (p // 128) * 2 + hh
        order = np.argsort(key, kind='stable')
        per_core.append((nodes[order], p[order], key[order]))
        cnt_all[c] = np.bincount(key, minlength=GT * 2).reshape(GT, 2)
    NBG = (cnt_all.max(axis=0) + 127) // 128           # [GT, 2] blocks
    nblk_sp = [int(NBG[:, h].sum()) for h in (0, 1)]
    sslots = [n * 128 for n in nblk_sp]
    base = np.zeros((GT, 2), np.int64)
    for h in (0, 1):
        base[:, h] = np.cumsum(NBG[:, h]) - NBG[:, h]
    idx_w, pl_w = [], []
    for c in range(N_CORES):
        nd, p, key = per_core[c]
        bounds = np.searchsorted(key, np.arange(GT * 2 + 1))
        idx_flat = np.zeros(sslots[0] + sslots[1], np.int64)
        pl_flat = -np.ones(sslots[0] + sslots[1], np.float32)
        for gi in range(GT):
            for h in (0, 1):
                lo, hi = bounds[gi * 2 + h], bounds[gi * 2 + h + 1]
                n = hi - lo
                off = (0 if h == 0 else sslots[0]) + int(base[gi, h]) * 128
                idx_flat[off:off + n] = nd[lo:hi] - h * SPL
                pl_flat[off:off + n] = (p[lo:hi] % 128).astype(np.float32)
        idx_w.append(_wrap_idx(idx_flat))
        pl_w.append(np.ascontiguousarray(pl_flat.reshape(-1, 128).T))
    meta = dict(NBG=NBG, nblk_sp=nblk_sp, sslots=sslots)
    return idx_w, pl_w, meta


def _calls(total_slots, call):
    out = []
    o = 0
    while o < total_slots:
        n = min(call, total_slots - o)
        out.append((o, n))
        o += n
    return out


def build_program(cfg, emeta, smeta):
    from concourse import bacc, mybir, tile

    f32, i16 = mybir.dt.float32, mybir.dt.int16
    bf16 = mybir.dt.bfloat16
    H, AH, OD = cfg.H, cfg.AH, cfg.OD
    NSH, NT, NTP = cfg.NSH, cfg.NT, cfg.NTP
    SSH, ST, GT = cfg.SSH, cfg.ST, cfg.GT
    QR, CHR = cfg.QR, cfg.CHR
    SPLT, SPL = cfg.SPLT, cfg.SPL
    S, b0, nblk, slots = emeta['S'], emeta['b0'], emeta['nblk'], emeta['slots']
    NBG, nblk_sp, sslots = smeta['NBG'], smeta['nblk_sp'], smeta['sslots']
    NBLK = nblk[0] + nblk[1]

    nc = bacc.Bacc("TRN2", target_bir_lowering=False, debug=False,
                   num_devices=N_CORES, num_swdge_queues=4,
                   dynamic_dma_scratch_size=32768)

    def din(name, shape, dt=f32):
        return nc.dram_tensor(name, shape, dt, kind="ExternalInput")

    xT = din("xT", [cfg.IN, NSH])
    dinv_pp_in = din("dinv_pp", [128, NT])
    dinv_bc_in = din("dinv_bc", [H, NSH])
    iota_in = din("iota_in", [128, 128], bf16)
    ident_in = din("ident_in", [128, 128])
    idx_gcn = din("idx_gcn", [128, (slots[0] + slots[1]) // 16], i16)
    dl_gcn = din("dl_gcn", [128, NBLK], bf16)
    idx_spot = din("idx_spot", [128, (sslots[0] + sslots[1]) // 16], i16)
    pl_spot = din("pl_spot", [128, nblk_sp[0] + nblk_sp[1]], bf16)
    projW = din("projW", [cfg.IN, H])
    W1, W2 = din("W1", [H, H]), din("W2", [H, H])
    attnW1, attnW2 = din("attnW1", [H, AH]), din("attnW2", [AH, 1])
    mlpW1, mlpW2 = din("mlpW1", [H, H]), din("mlpW2", [H, OD])
    # per-feature params packed [96, n]: cols = proj_b, gn0(w,b,a),
    # gcn1_b, gn1(w,b,a), gcn2_b, gn2(w,b,a), mlp_b1, mlpgn(w,b,a)
    pf = din("pf", [H, 16])
    attn_b1 = din("attn_b1", [AH, 1])
    attn_b2 = din("attn_b2", [1, 1])
    mlp_b2 = din("mlp_b2", [OD, 1])
    out = nc.dram_tensor("out", [SSH, OD], f32, kind="ExternalOutput")
    DEBUG = os.environ.get('KERNEL_DEBUG', '0') == '1'
    if DEBUG:
        dbg_h0 = nc.dram_tensor("dbg_h0", [H, NSH], f32, kind="ExternalOutput")
        dbg_h1 = nc.dram_tensor("dbg_h1", [H, NSH], f32, kind="ExternalOutput")
        dbg_h2 = nc.dram_tensor("dbg_h2", [H, NSH], f32, kind="ExternalOutput")
        dbg_sc = nc.dram_tensor("dbg_sc", [1, NSH], f32, kind="ExternalOutput")
        dbg_ps = nc.dram_tensor("dbg_ps", [cfg.S, H + 1], f32,
                                kind="ExternalOutput")
        dbg_sp = nc.dram_tensor("dbg_sp", [H, ST * 128], f32,
                                kind="ExternalOutput")

    gcalls = [_calls(slots[0], cfg.CALL), _calls(slots[1], cfg.CALL)]
    scalls = [_calls(sslots[0], cfg.CALL), _calls(sslots[1], cfg.CALL)]

    with tile.TileContext(nc) as tc:
        with (
            tc.tile_pool(name="res", bufs=1) as res,       # persistent
            tc.tile_pool(name="gat", bufs=4) as gat,       # gather tiles
            tc.tile_pool(name="ohp", bufs=3) as ohp,       # one-hot tiles
            tc.tile_pool(name="stg", bufs=1) as stg,       # table staging
            tc.tile_pool(name="sst", bufs=2) as sst,       # small staging
            tc.tile_pool(name="mmp", bufs=2, space="PSUM") as mmp,
            tc.tile_pool(name="scp", bufs=4, space="PSUM") as scp,
            tc.tile_pool(name="dram", bufs=1, space="DRAM") as dram,
        ):
            # ---------- persistent SBUF ----------
            h = res.tile([128, NSH], f32, name="h_act")       # rows 0:H+1
            agg = res.tile([H, NSH], f32, name="agg")
            dinv_bc = res.tile([H, NSH], f32, name="dinv_bc")
            dinv_pp = res.tile([128, NT], f32, name="dinv_pp")
            iota = res.tile([128, 128], bf16, name="iota")
            ident = res.tile([128, 128], f32, name="ident")
            idxg = res.tile([128, (slots[0] + slots[1]) // 16], i16,
                            name="idxg")
            dlg = res.tile([128, NBLK], bf16, name="dlg")
            idxs_sp = res.tile([128, (sslots[0] + sslots[1]) // 16], i16,
                               name="idxs_sp")
            plsp = res.tile([128, nblk_sp[0] + nblk_sp[1]], bf16,
                            name="plsp")
            wproj = res.tile([cfg.IN, H], f32, name="wproj")
            w1 = res.tile([H, H], f32, name="w1")
            w2 = res.tile([H, H], f32, name="w2")
            wa1 = res.tile([H, AH], f32, name="wa1")
            wa2 = res.tile([AH, 1], f32, name="wa2")
            wm1 = res.tile([H, H], f32, name="wm1")
            wm2 = res.tile([H, OD], f32, name="wm2")
            pft = res.tile([H, 16], f32, name="pft")
            ab1 = res.tile([AH, 1], f32, name="ab1")
            ab2 = res.tile([1, 1], f32, name="ab2")
            mb2 = res.tile([OD, 1], f32, name="mb2")
            sq = res.tile([H, 512], f32, name="sq")           # square scratch
            vec = res.tile([H, 8], f32, name="vec")           # tiny vector math
            spot_fm = res.tile([H, ST * 128], f32, name="spot_fm")
            rs_sb = res.tile([128, ST, H + 1], f32, name="rs_sb")
            stage_sp = res.tile([128, GT, H + 1], f32, name="stage_sp")
            e_pp = res.tile([128, NT], f32, name="e_pp")

            for t_, s_ in ((dinv_pp, dinv_pp_in), (dinv_bc, dinv_bc_in),
                           (iota, iota_in), (ident, ident_in),
                           (idxg, idx_gcn), (dlg, dl_gcn),
                           (idxs_sp, idx_spot), (plsp, pl_spot),
                           (wproj, projW), (w1, W1), (w2, W2), (wa1, attnW1),
                           (wa2, attnW2), (wm1, mlpW1), (wm2, mlpW2),
                           (pft, pf), (ab1, attn_b1), (ab2, attn_b2),
                           (mb2, mlp_b2)):
                nc.sync.dma_start(t_[:], s_[:])

            # DRAM: tables + collective bounces
            tbl_own = [dram.tile([NTP, 128], bf16, name=f"tblo{i}")
                       for i in range(2)]
            tbl_ch = [[dram.tile([CHR[hph], 128], bf16, addr_space="Shared",
                                 name=f"tblf{i}_{hph}") for hph in range(2)]
                      for i in range(2)]
            tbl_sp = dram.tile([NTP, 128], bf16, name="tblsp")
            partial_sp = dram.tile([cfg.S, H + 1], f32, name="partialsp")
            rs_out = dram.tile([SSH, H + 1], f32, name="rsout")
            st_in = [dram.tile([H, 2], f32, name=f"sti{i}") for i in range(4)]
            st_out = [dram.tile([H, 2], f32, addr_space="Shared",
                                name=f"sto{i}") for i in range(4)]

            NCHUNK = (NSH + 511) // 512

            def tsz(t):
                return min(128, NSH - t * 128)

            def csz(ci):
                return min(512, NSH - ci * 512)

            def graph_norm_relu(dst_ap, u_ap, width, n_total, stats_idx,
                                pre_b_col, gn_cols, parts=None):
                """dst = relu(S*u + B) with GN stats over u[:, :width].

                u is the pre-GN input WITHOUT the preceding linear bias
                (pre_b_col, a pf column or None); stats/affine fold it in.
                parts = (s1_parts, s2_parts) tiles of per-tile partial
                sums/square-sums already accumulated during the scatter.
                """
                s1 = vec[:, 0:1]
                if parts is not None:
                    s1p, s2p_ = parts
                    nc.vector.tensor_reduce(s1, s1p[:],
                                            mybir.AxisListType.X,
                                            mybir.AluOpType.add)
                    nc.vector.tensor_reduce(vec[:, 1:2], s2p_[:],
                                            mybir.AxisListType.X,
                                            mybir.AluOpType.add)
                else:
                    nc.vector.tensor_reduce(s1, u_ap[:, :width],
                                            mybir.AxisListType.X,
                                            mybir.AluOpType.add)
                    nch = (width + 511) // 512
                    s2p = res.tile([H, nch], f32, name=f"s2p{stats_idx}")
                    for ci in range(nch):
                        w_ = min(512, width - ci * 512)
                        nc.scalar.activation(
                            sq[:, :w_], u_ap[:, ci * 512:ci * 512 + w_],
                            mybir.ActivationFunctionType.Square,
                            accum_out=s2p[:, ci:ci + 1])
                    nc.vector.tensor_reduce(vec[:, 1:2], s2p[:],
                                            mybir.AxisListType.X,
                                            mybir.AluOpType.add)
                stv = sst.tile([H, 2], f32, name=f"stv{stats_idx}")
                nc.vector.tensor_copy(stv[:], vec[:, 0:2])
                nc.sync.dma_start(st_in[stats_idx][:], stv[:])
                nc.gpsimd.collective_compute(
                    "AllReduce", mybir.AluOpType.add,
                    replica_groups=[list(range(N_CORES))],
                    ins=[st_in[stats_idx][:].opt()],
                    outs=[st_out[stats_idx][:].opt()])
                stt = sst.tile([H, 2], f32, name=f"stt{stats_idx}")
                nc.sync.dma_start(stt[:], st_out[stats_idx][:])
                gw = pft[:, gn_cols[0]:gn_cols[0] + 1]
                gb = pft[:, gn_cols[1]:gn_cols[1] + 1]
                ga = pft[:, gn_cols[2]:gn_cols[2] + 1]
                mean = vec[:, 2:3]
                ex2 = vec[:, 3:4]
                inv_n = 1.0 / float(n_total)
                nc.vector.tensor_scalar(mean, stt[:, 0:1], inv_n, None,
                                        mybir.AluOpType.mult)
                nc.vector.tensor_scalar(ex2, stt[:, 1:2], inv_n, None,
                                        mybir.AluOpType.mult)
                if pre_b_col is not None:
                    c_ = pft[:, pre_b_col:pre_b_col + 1]
                    # mean_x = mean + c ; ex2_x = ex2 + 2*c*mean + c^2
                    t0 = vec[:, 4:5]
                    nc.vector.tensor_tensor(t0, c_, mean, mybir.AluOpType.mult)
                    nc.vector.tensor_scalar(t0, t0, 2.0, None,
                                            mybir.AluOpType.mult)
                    nc.vector.tensor_tensor(ex2, ex2, t0, mybir.AluOpType.add)
                    t1 = vec[:, 5:6]
                    nc.vector.tensor_tensor(t1, c_, c_, mybir.AluOpType.mult)
                    nc.vector.tensor_tensor(ex2, ex2, t1, mybir.AluOpType.add)
                    nc.vector.tensor_tensor(mean, mean, c_, mybir.AluOpType.add)
                # var = ex2 - mean^2 * a * (2 - a)
                m2 = vec[:, 4:5]
                nc.vector.tensor_tensor(m2, mean, mean, mybir.AluOpType.mult)
                a2 = vec[:, 5:6]
                nc.vector.tensor_scalar(a2, ga, -1.0, 2.0,
                                        mybir.AluOpType.mult,
                                        mybir.AluOpType.add)  # 2 - a
                nc.vector.tensor_tensor(a2, a2, ga, mybir.AluOpType.mult)
                nc.vector.tensor_tensor(m2, m2, a2, mybir.AluOpType.mult)
                var = vec[:, 6:7]
                nc.vector.tensor_tensor(var, ex2, m2,
                                        mybir.AluOpType.subtract)
                nc.vector.tensor_scalar(var, var, float(cfg.EPS), None,
                                        mybir.AluOpType.add)
                nc.scalar.activation(var, var,
                                     mybir.ActivationFunctionType.Sqrt)
                nc.vector.reciprocal(var, var)               # rs
                Sg = vec[:, 4:5]
                nc.vector.tensor_tensor(Sg, gw, var, mybir.AluOpType.mult)
                Bg = vec[:, 5:6]
                nc.vector.tensor_tensor(Bg, Sg, ga, mybir.AluOpType.mult)
                nc.vector.tensor_tensor(Bg, Bg, mean, mybir.AluOpType.mult)
                nc.vector.tensor_tensor(Bg, gb, Bg, mybir.AluOpType.subtract)
                if pre_b_col is not None:
                    c_ = pft[:, pre_b_col:pre_b_col + 1]
                    t0 = vec[:, 6:7]
                    nc.vector.tensor_tensor(t0, Sg, c_, mybir.AluOpType.mult)
                    nc.vector.tensor_tensor(Bg, Bg, t0, mybir.AluOpType.add)
                nc.scalar.activation(dst_ap, u_ap,
                                     mybir.ActivationFunctionType.Relu,
                                     bias=Bg, scale=Sg)

            # ================= proj layer =================
            nc.sync.dma_start(h[:cfg.IN, :NSH // 2], xT[:, :NSH // 2])
            nc.sync.dma_start(h[:cfg.IN, NSH // 2:], xT[:, NSH // 2:])
            for ci in range(NCHUNK):
                w_ = csz(ci)
                ps = mmp.tile([H, 512], f32, name=f"pj{ci}", tag="mm")
                nc.tensor.matmul(ps[:, :w_], wproj[:],
                                 h[:cfg.IN, ci * 512:ci * 512 + w_],
                                 start=True, stop=True)
                nc.vector.tensor_copy(agg[:, ci * 512:ci * 512 + w_],
                                      ps[:, :w_])
            # pf cols: 0=proj_b, (1,2,3)=gn0, 4=gcn1_b, (5,6,7)=gn1,
            #          8=gcn2_b, (9,10,11)=gn2, 12=mlp_b1, (13,14,15)=mlpgn
            graph_norm_relu(h[:H, :], agg[:], NSH, cfg.N, 0, 0, (1, 2, 3))
            if DEBUG:
                nc.sync.dma_start(dbg_h0[:], h[:H, :])

            # ================= GCN layers =================
            qn = 0
            for li, (Wt, b_col, gn_cols) in enumerate(
                    ((w1, 4, (5, 6, 7)), (w2, 8, (9, 10, 11)))):
                # node-major bf16 table t' = dinv * (h @ W), staged in two
                # separate tiles so piece-0's DMA + AllGather overlap the
                # build of piece-1 tiles (tile-granular dependency tracking)
                sg0 = stg.tile([128, SPLT, 128], bf16, name=f"sg0_{li}",
                               tag="ts0")
                sg1 = stg.tile([128, NT - SPLT, 128], bf16, name=f"sg1_{li}",
                               tag="ts1")
                nc.vector.memset(sg0[:], 0.0)
                nc.vector.memset(sg1[:], 0.0)
                for t in range(NT):
                    n_ = tsz(t)
                    ps = mmp.tile([128, H], f32, name=f"tb{li}_{t}", tag="mm")
                    nc.tensor.matmul(ps[:n_, :], h[:H, t * 128:t * 128 + n_],
                                     Wt[:], start=True, stop=True)
                    sgd = sg0[:n_, t, :H] if t < SPLT else \
                        sg1[:n_, t - SPLT, :H]
                    nc.vector.tensor_scalar(sgd, ps[:n_, :],
                                            dinv_pp[:n_, t:t + 1], None,
                                            mybir.AluOpType.mult)
                nc.sync.dma_start(
                    tbl_own[li][:SPL].rearrange("(t p) e -> p t e", p=128),
                    sg0[:])
                nc.sync.dma_start(
                    tbl_own[li][SPL:].rearrange("(t p) e -> p t e", p=128),
                    sg1[:])
                for hph in (0, 1):
                    iv = tbl_own[li][:SPL, :] if hph == 0 else \
                        tbl_own[li][SPL:, :]
                    nc.gpsimd.collective_compute(
                        "AllGather", mybir.AluOpType.bypass,
                        replica_groups=[list(range(N_CORES))],
                        ins=[iv.opt()], outs=[tbl_ch[li][hph][:].opt()])
                # self-loop init: agg = dinv * (h @ W)
                for ci in range(NCHUNK):
                    w_ = csz(ci)
                    ps = mmp.tile([H, 512], f32, name=f"sf{li}_{ci}", tag="mm")
                    nc.tensor.matmul(ps[:, :w_], Wt[:],
                                     h[:H, ci * 512:ci * 512 + w_],
                                     start=True, stop=True)
                    nc.vector.tensor_tensor(
                        agg[:, ci * 512:ci * 512 + w_], ps[:, :w_],
                        dinv_bc[:, ci * 512:ci * 512 + w_],
                        mybir.AluOpType.mult)
                # gather + scatter, two phases (AG chunks)
                s1p = res.tile([H, NT], f32, name=f"s1p{li}")
                s2p = res.tile([H, NT], f32, name=f"s2p_{li}")
                for hph in (0, 1):
                    tview = tbl_ch[li][hph][:]
                    col0 = 0 if hph == 0 else slots[0] // 16
                    blk0 = 0 if hph == 0 else nblk[0]
                    tile_of = np.repeat(np.arange(NT), S[:, hph])
                    open_ps = None
                    open_t = -1
                    for k, (o, n) in enumerate(gcalls[hph]):
                        nb = n // 128
                        g = gat.tile([128, cfg.CALL // 128, 128], bf16,
                                     name=f"g{li}_{hph}_{k}", tag="gat")
                        nc.gpsimd.dma_gather(
                            g[:, :nb, :], tview,
                            idxg[:, col0 + o // 16: col0 + (o + n) // 16],
                            n, n, 128, single_packet=False, queue_num=qn)
                        qn = (qn + 1) % 4
                        oh = ohp.tile([128, cfg.CALL // 128, 128], bf16,
                                      name=f"oh{li}_{hph}_{k}", tag="oh")
                        dlsl = dlg[:, blk0 + o // 128: blk0 + (o + n) // 128]
                        nc.vector.tensor_tensor(
                            oh[:, :nb, :],
                            iota[:].unsqueeze(1).broadcast_to([128, nb, 128]),
                            dlsl.unsqueeze(2).broadcast_to([128, nb, 128]),
                            mybir.AluOpType.is_equal)
                        for j in range(nb):
                            b = o // 128 + j
                            t = int(tile_of[b])
                            if t != open_t:
                                open_ps = scp.tile([H, 128], f32,
                                                   name=f"sc{li}_{hph}_{b}",
                                                   tag="sc")
                                open_t = t
                                first = True
                            else:
                                first = False
                            last = (b + 1 == len(tile_of)) or \
                                   (tile_of[b + 1] != t)
                            nc.tensor.matmul(open_ps[:], g[:, j, :H],
                                             oh[:, j, :],
                                             start=first, stop=last)
                            if last:
                                n_ = tsz(t)
                                av = agg[:, t * 128:t * 128 + n_]
                                nc.vector.tensor_tensor(
                                    av, av, open_ps[:, :n_],
                                    mybir.AluOpType.add)
                                if hph == 1:
                                    # finalize tile: u = agg*dinv, partial
                                    # GN stats while the scatter continues
                                    nc.vector.tensor_tensor(
                                        av, av,
                                        dinv_bc[:, t * 128:t * 128 + n_],
                                        mybir.AluOpType.mult)
                                    nc.vector.tensor_reduce(
                                        s1p[:, t:t + 1], av,
                                        mybir.AxisListType.X,
                                        mybir.AluOpType.add)
                                    nc.scalar.activation(
                                        sq[:, :n_], av,
                                        mybir.ActivationFunctionType.Square,
                                        accum_out=s2p[:, t:t + 1])
                graph_norm_relu(h[:H, :], agg[:], NSH, cfg.N,
                                1 + li, b_col, gn_cols, parts=(s1p, s2p))
                if DEBUG:
                    nc.sync.dma_start((dbg_h1 if li == 0 else dbg_h2)[:],
                                      h[:H, :])

            # ================= attention scores =================
            # u_att = relu(attn_W1.T @ h + b1); score = attn_W2.T @ u.
            # attn_b2 is dropped: softmax is shift-invariant, exp(b2)
            # cancels between numerator and denominator exactly.
            # e = exp(score) is laid out node-major [128, NT] so the spot
            # table build only depends on h (done) + this chunk's scores.
            for ci in range(NCHUNK):
                w_ = csz(ci)
                ps = mmp.tile([AH, 512], f32, name=f"at{ci}", tag="mm")
                nc.tensor.matmul(ps[:, :w_], wa1[:],
                                 h[:H, ci * 512:ci * 512 + w_],
                                 start=True, stop=True)
                uc = sst.tile([AH, 512], f32, name=f"uat{ci}", tag="uat")
                nc.scalar.activation(uc[:, :w_], ps[:, :w_],
                                     mybir.ActivationFunctionType.Relu,
                                     bias=ab1[:])
                ps2 = mmp.tile([1, 512], f32, name=f"sc2{ci}", tag="mm")
                nc.tensor.matmul(ps2[:, :w_], wa2[:], uc[:, :w_],
                                 start=True, stop=True)
                scb = sst.tile([1, 512], f32, name=f"scb{ci}", tag="uat")
                nc.vector.tensor_copy(scb[:, :w_], ps2[:, :w_])
                nbj = (w_ + 127) // 128
                for j in range(nbj):
                    wj = min(128, w_ - j * 128)
                    pt = mmp.tile([128, 1], f32, name=f"et{ci}_{j}",
                                  tag="tpose")
                    nc.tensor.transpose(pt[:wj, :1],
                                        scb[:1, j * 128:j * 128 + wj],
                                        ident[:1, :1])
                    nc.scalar.activation(
                        e_pp[:wj, 4 * ci + j:4 * ci + j + 1], pt[:wj, :1],
                        mybir.ActivationFunctionType.Exp)

            # ======== spot table rows: [e*h (H) | e | pad] bf16 ========
            sp0 = stg.tile([128, SPLT, 128], bf16, name="sgsp0", tag="ts0")
            sp1 = stg.tile([128, NT - SPLT, 128], bf16, name="sgsp1",
                           tag="ts1")
            nc.vector.memset(sp0[:], 0.0)
            nc.vector.memset(sp1[:], 0.0)
            for t in range(NT):
                n_ = tsz(t)
                ps = mmp.tile([128, H], f32, name=f"tr{t}", tag="tpose")
                nc.tensor.transpose(ps[:n_, :H],
                                    h[:H, t * 128:t * 128 + n_],
                                    ident[:H, :H])
                spd = sp0 if t < SPLT else sp1
                td = t if t < SPLT else t - SPLT
                nc.vector.tensor_scalar(spd[:n_, td, :H], ps[:n_, :H],
                                        e_pp[:n_, t:t + 1], None,
                                        mybir.AluOpType.mult)
                nc.vector.tensor_copy(spd[:n_, td, H:H + 1],
                                      e_pp[:n_, t:t + 1])
            nc.sync.dma_start(
                tbl_sp[:SPL].rearrange("(t p) e -> p t e", p=128), sp0[:])
            nc.sync.dma_start(
                tbl_sp[SPL:].rearrange("(t p) e -> p t e", p=128), sp1[:])

            # ===== local spot scatter: partial sums per 128-spot group =====
            nc.vector.memset(stage_sp[:], 0.0)
            for sph in (0, 1):
                tview = tbl_sp[:cfg.SPL] if sph == 0 else tbl_sp[cfg.SPL:]
                col0 = 0 if sph == 0 else sslots[0] // 16
                blk0 = 0 if sph == 0 else nblk_sp[0]
                tile_of_sp = np.repeat(np.arange(GT), NBG[:, sph])
                open_ps = None
                open_g = -1
                for k, (o, n) in enumerate(scalls[sph]):
                    nb = n // 128
                    g = gat.tile([128, cfg.CALL // 128, 128], bf16,
                                 name=f"gs{sph}_{k}", tag="gat")
                    nc.gpsimd.dma_gather(
                        g[:, :nb, :], tview,
                        idxs_sp[:, col0 + o // 16:col0 + (o + n) // 16],
                        n, n, 128, single_packet=False, queue_num=qn)
                    qn = (qn + 1) % 4
                    oh = ohp.tile([128, cfg.CALL // 128, 128], bf16,
                                  name=f"ohs{sph}_{k}", tag="oh")
                    plsl = plsp[:, blk0 + o // 128:blk0 + (o + n) // 128]
                    nc.vector.tensor_tensor(
                        oh[:, :nb, :],
                        iota[:].unsqueeze(1).broadcast_to([128, nb, 128]),
                        plsl.unsqueeze(2).broadcast_to([128, nb, 128]),
                        mybir.AluOpType.is_equal)
                    for j in range(nb):
                        b = o // 128 + j
                        gi = int(tile_of_sp[b])
                        if gi != open_g:
                            open_ps = scp.tile([128, H + 1], f32,
                                               name=f"scs{sph}_{b}", tag="sc")
                            open_g = gi
                            first = True
                        else:
                            first = False
                        last = (b + 1 == len(tile_of_sp)) or \
                               (tile_of_sp[b + 1] != gi)
                        nc.tensor.matmul(open_ps[:], oh[:, j, :],
                                         g[:, j, :H + 1],
                                         start=first, stop=last)
                        if last:
                            nc.vector.tensor_tensor(
                                stage_sp[:, gi, :], stage_sp[:, gi, :],
                                open_ps[:], mybir.AluOpType.add)
            gfull = (cfg.S // 128) * 128
            nc.sync.dma_start(
                partial_sp[:gfull, :].rearrange("(g p) f -> p g f", p=128),
                stage_sp[:, :cfg.S // 128, :])
            if cfg.S % 128:
                nc.sync.dma_start(partial_sp[gfull:, :],
                                  stage_sp[:cfg.S % 128, GT - 1, :])
            if DEBUG:
                nc.sync.dma_start(dbg_ps[:], partial_sp[:])
            nc.gpsimd.collective_compute(
                "ReduceScatter", mybir.AluOpType.add,
                replica_groups=[list(range(N_CORES))],
                ins=[partial_sp[:].opt()], outs=[rs_out[:].opt()])

            # ===== normalize spot vectors, back to feature-major =====
            nc.vector.memset(spot_fm[:], 0.0)
            for gi in range(ST):
                gsz = min(128, SSH - gi * 128)
                nc.sync.dma_start(rs_sb[:gsz, gi, :],
                                  rs_out[gi * 128:gi * 128 + gsz, :])
                den = sst.tile([128, 1], f32, name=f"den{gi}", tag="den")
                nc.vector.tensor_scalar(den[:gsz], rs_sb[:gsz, gi, H:H + 1],
                                        1e-30, None, mybir.AluOpType.max)
                nc.vector.reciprocal(den[:gsz], den[:gsz])
                nc.vector.tensor_scalar(rs_sb[:gsz, gi, :H],
                                        rs_sb[:gsz, gi, :H],
                                        den[:gsz], None,
                                        mybir.AluOpType.mult)
                ps = mmp.tile([H, 128], f32, name=f"spt{gi}", tag="tpose")
                nc.tensor.transpose(ps[:, :gsz], rs_sb[:gsz, gi, :H],
                                    ident[:gsz, :gsz])
                nc.vector.tensor_copy(spot_fm[:, gi * 128:gi * 128 + gsz],
                                      ps[:, :gsz])
            if DEBUG:
                nc.sync.dma_start(dbg_sp[:], spot_fm[:])

            # ================= MLP head =================
            um = res.tile([H, ST * 128], f32, name="um")
            for ci in range((ST * 128 + 511) // 512):
                w_ = min(512, ST * 128 - ci * 512)
                ps = mmp.tile([H, 512], f32, name=f"m1{ci}", tag="mm")
                nc.tensor.matmul(ps[:, :w_], wm1[:],
                                 spot_fm[:, ci * 512:ci * 512 + w_],
                                 start=True, stop=True)
                nc.vector.tensor_copy(um[:, ci * 512:ci * 512 + w_],
                                      ps[:, :w_])
            graph_norm_relu(um[:], um[:], SSH, cfg.S, 3, 12, (13, 14, 15))
            zo = res.tile([OD, ST * 128], f32, name="zo")
            for ci in range((ST * 128 + 511) // 512):
                w_ = min(512, ST * 128 - ci * 512)
                ps = mmp.tile([OD, 512], f32, name=f"m2{ci}", tag="mm")
                nc.tensor.matmul(ps[:, :w_], wm2[:],
                                 um[:, ci * 512:ci * 512 + w_],
                                 start=True, stop=True)
                nc.vector.tensor_scalar(zo[:, ci * 512:ci * 512 + w_],
                                        ps[:, :w_], mb2[:], None,
                                        mybir.AluOpType.add)
            for gi in range(ST):
                n_ = min(128, SSH - gi * 128)
                if n_ <= 0:
                    break
                ps = mmp.tile([128, OD], f32, name=f"ot{gi}", tag="tpose")
                nc.tensor.transpose(ps[:, :], zo[:, gi * 128:(gi + 1) * 128],
                                    ident[:OD, :OD])
                sg = sst.tile([128, OD], f32, name=f"os{gi}", tag="ostg")
                nc.vector.tensor_copy(sg[:], ps[:])
                nc.sync.dma_start(out[gi * 128:gi * 128 + n_, :], sg[:n_, :])

    nc.compile()
    return nc


_CACHE = {}


def _build_inputs(cfg, inputs, idx_w, dl_w, idxs_w, pl_w, dinv):
    f = np.float32
    x = np.asarray(inputs['x'], f)
    xT = np.ascontiguousarray(x.T)

    def col(v):
        return np.asarray(v, f).reshape(-1, 1)

    pf = np.zeros((cfg.H, 16), f)
    for i, k in enumerate(['proj_b', 'gn0_w', 'gn0_b', 'gn0_a',
                           'gcn1_b', 'gn1_w', 'gn1_b', 'gn1_a',
                           'gcn2_b', 'gn2_w', 'gn2_b', 'gn2_a',
                           'mlp_b1', 'mlp_gn_w', 'mlp_gn_b', 'mlp_gn_a']):
        pf[:, i] = np.asarray(inputs[k], f)
    import ml_dtypes
    iota = np.broadcast_to(np.arange(128, dtype=f),
                           (128, 128)).astype(ml_dtypes.bfloat16)
    ident = np.eye(128, dtype=f)
    in_maps = []
    for c in range(N_CORES):
        n0 = c * cfg.NSH
        dinv_own = dinv[n0:n0 + cfg.NSH]
        dpp = np.ones((128, cfg.NT), f)
        for t in range(cfg.NT):
            n_ = min(128, cfg.NSH - t * 128)
            dpp[:n_, t] = dinv_own[t * 128:t * 128 + n_]
        dbc = np.broadcast_to(dinv_own[None, :], (cfg.H, cfg.NSH)).copy()
        in_maps.append({
            'xT': np.ascontiguousarray(xT[:, n0:n0 + cfg.NSH]),
            'dinv_pp': dpp, 'dinv_bc': dbc,
            'iota_in': iota, 'ident_in': ident,
            'idx_gcn': idx_w[c],
            'dl_gcn': dl_w[c].astype(ml_dtypes.bfloat16),
            'idx_spot': idxs_w[c],
            'pl_spot': pl_w[c].astype(ml_dtypes.bfloat16),
            'projW': np.asarray(inputs['proj_W'], f),
            'W1': np.asarray(inputs['gcn1_W'], f),
            'W2': np.asarray(inputs['gcn2_W'], f),
            'attnW1': np.asarray(inputs['attn_W1'], f),
            'attnW2': np.asarray(inputs['attn_W2'], f),
            'mlpW1': np.asarray(inputs['mlp_W1'], f),
            'mlpW2': np.asarray(inputs['mlp_W2'], f),
            'pf': pf,
            'attn_b1': col(inputs['attn_b1']),
            'attn_b2': col(inputs['attn_b2']),
            'mlp_b2': col(inputs['mlp_b2']),
        })
    return in_maps


def kernel(**inputs):
    from concourse import bass_utils
    cfg = Cfg(n_nodes=int(np.asarray(inputs['x']).shape[0]),
              n_edges=int(np.asarray(inputs['edge_index']).shape[1]),
              in_dim=int(np.asarray(inputs['x']).shape[1]),
              hid=int(np.asarray(inputs['proj_W']).shape[1]),
              attn_hid=int(np.asarray(inputs['attn_W1']).shape[1]),
              out_dim=int(np.asarray(inputs['mlp_W2']).shape[1]),
              n_spots=int(inputs['num_spots']))
    ei = np.asarray(inputs['edge_index']).astype(np.int64)
    cts = np.asarray(inputs['cell_to_spot']).astype(np.int64)
    src, dst = ei[0], ei[1]
    deg = (np.bincount(dst, minlength=cfg.N) + 1).astype(np.float64)
    dinv = (1.0 / np.sqrt(deg)).astype(np.float32)

    idx_w, dl_w, emeta = _prep_edges(cfg, src, dst)
    idxs_w, pl_w, smeta = _prep_spots(cfg, cts)

    key = (cfg.N, cfg.E, tuple(emeta['nblk']), tuple(smeta['nblk_sp']))
    if key not in _CACHE:
        _CACHE[key] = build_program(cfg, emeta, smeta)
    nc = _CACHE[key]

    in_maps = _build_inputs(cfg, inputs, idx_w, dl_w, idxs_w, pl_w, dinv)
    res = bass_utils.run_bass_kernel_spmd(
        nc, in_maps, core_ids=list(range(N_CORES)),
        trace=os.environ.get('KERNEL_TRACE', '0') == '1',
        tmpdir=os.environ.get('KERNEL_TMPD'))
    if os.environ.get('KERNEL_TRACE', '0') == '1':
        print('HW exec time:', res.exec_time_ns, 'ns')
    out = np.concatenate([res.results[c]['out'] for c in range(N_CORES)],
                         axis=0)
    return out.astype(np.float32)


# revision 54
# speedup vs baseline: 1.0578x; 1.0578x over previous
"""GCN pipeline (proj + 2x GCNConv + GraphNorm + spot-softmax aggregation +
MLP head) on 8 trn2 NeuronCores via Bass/Tile.

Sharding: core c owns nodes [c*NSH,(c+1)*NSH) and spots [c*SSH,(c+1)*SSH).
Activations are feature-major [H, NSH] in SBUF.

GCN layers: per-layer bf16 node-major table t' = dinv*(h@W) (128 bf16/row =
256B) is AllGathered; dst-sorted edges gather rows by src (dma_gather, 4 SWDGE
queues) and are scatter-accumulated per 128-dst tile with one-hot matmuls on
the PE (stationary = gathered rows, moving = one-hot, psum accumulates
feature-major [H,128] added straight into agg).

Spot aggregation: softmax without max-subtraction (scores are O(1)); each core
builds local rows [e*h | e] bf16 (e = exp(score)), gathers them in
spot-sorted order from local HBM, PE-scatters into per-128-spot-group partial
sums [S,97] f32, then one ReduceScatter(add) gives each core its own spot
shard; normalize by the e-sum column and run the MLP head locally.
"""
import sys, os
sys.path.insert(0, '/opt/trn_rl_repo')
import numpy as np

N_CORES = 8


class Cfg:
    def __init__(self, n_nodes=50000, n_edges=800000, in_dim=128, hid=96,
                 attn_hid=32, out_dim=16, n_spots=5000, eps=1e-5):
        assert n_nodes % N_CORES == 0 and n_spots % N_CORES == 0
        self.N, self.E, self.IN, self.H = n_nodes, n_edges, in_dim, hid
        self.AH, self.OD, self.S, self.EPS = attn_hid, out_dim, n_spots, eps
        self.NSH = n_nodes // N_CORES
        self.SSH = n_spots // N_CORES
        self.NT = (self.NSH + 127) // 128          # node tiles per core
        self.NTP = self.NT * 128                   # padded rows per core
        self.SPLT = self.NT // 2                   # table piece-0 tiles
        self.SPL = self.SPLT * 128                 # table split row
        self.QR = [self.SPL, self.NTP - self.SPL]  # AG chunk rows per core
        self.CHR = [N_CORES * q for q in self.QR]  # chunk table rows
        assert max(self.CHR) < (1 << 15)
        self.ST = (self.SSH + 127) // 128          # spot groups per core
        self.GT = (n_spots + 127) // 128           # global spot groups
        self.TROW = 128                            # table row elems (bf16)
        self.CALL = 2048                           # gather slots per call


def _wrap_idx(flat):
    """int16 slot list -> [128, n/16] wrapped layout (replicated 8x)."""
    n = len(flat)
    assert n % 16 == 0
    w = flat.reshape(n // 16, 16).T.astype(np.int16)   # [16, n/16]
    return np.tile(w, (8, 1))


def _prep_edges(cfg, src, dst):
    """Per-core dst-sorted edge structure, uniform across cores for SPMD.

    Phase h covers edges whose src local row is in AG chunk h (local rows
    [h*QROWS,(h+1)*QROWS)); chunk-h table = concat over cores of that row
    range, so phase-0 gathers can start when the first chunk AllGather lands.
    Within a phase: tiles 0..NT-1, S[t,h] blocks each. Returns per-core
    wrapped idx arrays, per-block dst-local columns, and shared meta.
    """
    NSH, NT = cfg.NSH, cfg.NT
    Q0, Q1 = cfg.QR
    src_core = src // NSH
    src_r = src % NSH                                  # local row (= node id)
    core_of = dst // NSH
    per_core = []
    cnt_all = np.zeros((N_CORES, NT, 2), np.int64)
    for c in range(N_CORES):
        m = core_of == c
        h_c = (src_r[m] >= Q0).astype(np.int64)
        r_c = np.where(h_c == 1,
                       src_core[m] * Q1 + src_r[m] - Q0,
                       src_core[m] * Q0 + src_r[m])    # row within chunk
        d_c = dst[m] - c * NSH
        t_c = d_c // 128
        key = t_c * 2 + h_c
        order = np.argsort(key, kind='stable')
        per_core.append((r_c[order], d_c[order], key[order]))
        cnt_all[c] = np.bincount(key, minlength=NT * 2).reshape(NT, 2)
    S = (cnt_all.max(axis=0) + 127) // 128             # [NT, 2] blocks
    assert S.min() >= 1      # every tile closes in phase 1 (finalization)
    nblk = [int(S[:, h].sum()) for h in (0, 1)]
    slots = [n * 128 for n in nblk]
    b0 = np.zeros((NT, 2), np.int64)
    for h in (0, 1):
        b0[:, h] = np.cumsum(S[:, h]) - S[:, h]
    idx_w, dl_w = [], []
    for c in range(N_CORES):
        r_c, d_c, key = per_core[c]
        bounds = np.searchsorted(key, np.arange(NT * 2 + 1))
        idx_flat = np.zeros(slots[0] + slots[1], np.int64)
        dl_flat = -np.ones(slots[0] + slots[1], np.float32)
        for t in range(NT):
            for h in (0, 1):
                lo, hi = bounds[t * 2 + h], bounds[t * 2 + h + 1]
                n = hi - lo
                off = (0 if h == 0 else slots[0]) + int(b0[t, h]) * 128
                idx_flat[off:off + n] = r_c[lo:hi]
                dl_flat[off:off + n] = (d_c[lo:hi] % 128).astype(np.float32)
        idx_w.append(_wrap_idx(idx_flat))
        dl_w.append(np.ascontiguousarray(
            dl_flat.reshape(-1, 128).T))               # [128, nblk_tot]
    meta = dict(S=S, b0=b0, nblk=nblk, slots=slots)
    return idx_w, dl_w, meta


def _prep_spots(cfg, cts):
    """Per-core spot-sorted local-node structure for the partial-sum scatter.

    Local nodes sorted by global spot id, split into 128-slot blocks aligned
    to 128-spot groups (group g covers spots [g*128,(g+1)*128)). Uniform
    block counts (max over cores) for SPMD.
    """
    NSH, GT, SPL = cfg.NSH, cfg.GT, cfg.SPL
    nodes = np.arange(NSH)
    per_core = []
    cnt_all = np.zeros((N_CORES, GT, 2), np.int64)
    for c in range(N_CORES):
        p = cts[c * NSH:(c + 1) * NSH]
        hh = (nodes >= SPL).astype(np.int64)
        key = (p // 128) * 2 + hh
        order = np.argsort(key, kind='stable')
        per_core.append((nodes[order], p[order], key[order]))
        cnt_all[c] = np.bincount(key, minlength=GT * 2).reshape(GT, 2)
    NBG = (cnt_all.max(axis=0) + 127) // 128           # [GT, 2] blocks
    nblk_sp = [int(NBG[:, h].sum()) for h in (0, 1)]
    sslots = [n * 128 for n in nblk_sp]
    base = np.zeros((GT, 2), np.int64)
    for h in (0, 1):
        base[:, h] = np.cumsum(NBG[:, h]) - NBG[:, h]
    idx_w, pl_w = [], []
    for c in range(N_CORES):
        nd, p, key = per_core[c]
        bounds = np.searchsorted(key, np.arange(GT * 2 + 1))
        idx_flat = np.zeros(sslots[0] + sslots[1], np.int64)
        pl_flat = -np.ones(sslots[0] + sslots[1], np.float32)
        for gi in range(GT):
            for h in (0, 1):
                lo, hi = bounds[gi * 2 + h], bounds[gi * 2 + h + 1]
                n = hi - lo
                off = (0 if h == 0 else sslots[0]) + int(base[gi, h]) * 128
                idx_flat[off:off + n] = nd[lo:hi] - h * SPL
                pl_flat[off:off + n] = (p[lo:hi] % 128).astype(np.float32)
        idx_w.append(_wrap_idx(idx_flat))
        pl_w.append(np.ascontiguousarray(pl_flat.reshape(-1, 128).T))
    meta = dict(NBG=NBG, nblk_sp=nblk_sp, sslots=sslots)
    return idx_w, pl_w, meta


def _calls(total_slots, call):
    out = []
    o = 0
    while o < total_slots:
        n = min(call, total_slots - o)
        out.append((o, n))
        o += n
    return out


def build_program(cfg, emeta, smeta):
    from concourse import bacc, mybir, tile

    f32, i16 = mybir.dt.float32, mybir.dt.int16
    bf16 = mybir.dt.bfloat16
    H, AH, OD = cfg.H, cfg.AH, cfg.OD
    NSH, NT, NTP = cfg.NSH, cfg.NT, cfg.NTP
    SSH, ST, GT = cfg.SSH, cfg.ST, cfg.GT
    QR, CHR = cfg.QR, cfg.CHR
    SPLT, SPL = cfg.SPLT, cfg.SPL
    S, b0, nblk, slots = emeta['S'], emeta['b0'], emeta['nblk'], emeta['slots']
    NBG, nblk_sp, sslots = smeta['NBG'], smeta['nblk_sp'], smeta['sslots']
    NBLK = nblk[0] + nblk[1]

    nc = bacc.Bacc("TRN2", target_bir_lowering=False, debug=False,
                   num_devices=N_CORES, num_swdge_queues=4,
                   dynamic_dma_scratch_size=32768)

    def din(name, shape, dt=f32):
        return nc.dram_tensor(name, shape, dt, kind="ExternalInput")

    xT = din("xT", [cfg.IN, NSH])
    dinv_pp_in = din("dinv_pp", [128, NT])
    dinv_bc_in = din("dinv_bc", [H, NSH])
    iota_in = din("iota_in", [128, 128], bf16)
    ident_in = din("ident_in", [128, 128])
    idx_gcn = din("idx_gcn", [128, (slots[0] + slots[1]) // 16], i16)
    dl_gcn = din("dl_gcn", [128, NBLK], bf16)
    idx_spot = din("idx_spot", [128, (sslots[0] + sslots[1]) // 16], i16)
    pl_spot = din("pl_spot", [128, nblk_sp[0] + nblk_sp[1]], bf16)
    projW = din("projW", [cfg.IN, H])
    W1, W2 = din("W1", [H, H]), din("W2", [H, H])
    attnW1, attnW2 = din("attnW1", [H, AH]), din("attnW2", [AH, 1])
    mlpW1, mlpW2 = din("mlpW1", [H, H]), din("mlpW2", [H, OD])
    # per-feature params packed [96, n]: cols = proj_b, gn0(w,b,a),
    # gcn1_b, gn1(w,b,a), gcn2_b, gn2(w,b,a), mlp_b1, mlpgn(w,b,a)
    pf = din("pf", [H, 16])
    attn_b1 = din("attn_b1", [AH, 1])
    attn_b2 = din("attn_b2", [1, 1])
    mlp_b2 = din("mlp_b2", [OD, 1])
    out = nc.dram_tensor("out", [SSH, OD], f32, kind="ExternalOutput")
    DEBUG = os.environ.get('KERNEL_DEBUG', '0') == '1'
    if DEBUG:
        dbg_h0 = nc.dram_tensor("dbg_h0", [H, NSH], f32, kind="ExternalOutput")
        dbg_h1 = nc.dram_tensor("dbg_h1", [H, NSH], f32, kind="ExternalOutput")
        dbg_h2 = nc.dram_tensor("dbg_h2", [H, NSH], f32, kind="ExternalOutput")
        dbg_sc = nc.dram_tensor("dbg_sc", [1, NSH], f32, kind="ExternalOutput")
        dbg_ps = nc.dram_tensor("dbg_ps", [cfg.S, H + 1], f32,
                                kind="ExternalOutput")
        dbg_sp = nc.dram_tensor("dbg_sp", [H, ST * 128], f32,
                                kind="ExternalOutput")

    gcalls = [_calls(slots[0], cfg.CALL), _calls(slots[1], cfg.CALL)]
    scalls = [_calls(sslots[0], cfg.CALL), _calls(sslots[1], cfg.CALL)]

    with tile.TileContext(nc) as tc:
        with (
            tc.tile_pool(name="res", bufs=1) as res,       # persistent
            tc.tile_pool(name="gat", bufs=5) as gat,       # gather tiles
            tc.tile_pool(name="ohp", bufs=3) as ohp,       # one-hot tiles
            tc.tile_pool(name="stg", bufs=1) as stg,       # table staging
            tc.tile_pool(name="sst", bufs=3) as sst,       # small staging
            tc.tile_pool(name="mmp", bufs=2, space="PSUM") as mmp,
            tc.tile_pool(name="scp", bufs=4, space="PSUM") as scp,
            tc.tile_pool(name="dram", bufs=1, space="DRAM") as dram,
        ):
            # ---------- persistent SBUF ----------
            h = res.tile([128, NSH], f32, name="h_act")       # rows 0:H+1
            agg = res.tile([H, NSH], f32, name="agg")
            dinv_bc = res.tile([H, NSH], f32, name="dinv_bc")
            dinv_pp = res.tile([128, NT], f32, name="dinv_pp")
            iota = res.tile([128, 128], bf16, name="iota")
            ident = res.tile([128, 128], f32, name="ident")
            idxg = res.tile([128, (slots[0] + slots[1]) // 16], i16,
                            name="idxg")
            dlg = res.tile([128, NBLK], bf16, name="dlg")
            idxs_sp = res.tile([128, (sslots[0] + sslots[1]) // 16], i16,
                               name="idxs_sp")
            plsp = res.tile([128, nblk_sp[0] + nblk_sp[1]], bf16,
                            name="plsp")
            wproj = res.tile([cfg.IN, H], f32, name="wproj")
            w1 = res.tile([H, H], f32, name="w1")
            w2 = res.tile([H, H], f32, name="w2")
            wa1 = res.tile([H, AH], f32, name="wa1")
            wa2 = res.tile([AH, 1], f32, name="wa2")
            wm1 = res.tile([H, H], f32, name="wm1")
            wm2 = res.tile([H, OD], f32, name="wm2")
            pft = res.tile([H, 16], f32, name="pft")
            ab1 = res.tile([AH, 1], f32, name="ab1")
            ab2 = res.tile([1, 1], f32, name="ab2")
            mb2 = res.tile([OD, 1], f32, name="mb2")
            sq = res.tile([H, 512], f32, name="sq")           # square scratch
            vec = res.tile([H, 8], f32, name="vec")           # tiny vector math
            spot_fm = res.tile([H, ST * 128], f32, name="spot_fm")
            rs_sb = res.tile([128, ST, H + 1], f32, name="rs_sb")
            stage_sp = res.tile([128, GT, H + 1], f32, name="stage_sp")
            e_pp = res.tile([128, NT], f32, name="e_pp")

            for t_, s_ in ((dinv_pp, dinv_pp_in), (dinv_bc, dinv_bc_in),
                           (iota, iota_in), (ident, ident_in),
                           (idxg, idx_gcn), (dlg, dl_gcn),
                           (idxs_sp, idx_spot), (plsp, pl_spot),
                           (wproj, projW), (w1, W1), (w2, W2), (wa1, attnW1),
                           (wa2, attnW2), (wm1, mlpW1), (wm2, mlpW2),
                           (pft, pf), (ab1, attn_b1), (ab2, attn_b2),
                           (mb2, mlp_b2)):
                nc.sync.dma_start(t_[:], s_[:])

            # DRAM: tables + collective bounces
            tbl_own = [dram.tile([NTP, 128], bf16, name=f"tblo{i}")
                       for i in range(2)]
            tbl_ch = [[dram.tile([CHR[hph], 128], bf16, addr_space="Shared",
                                 name=f"tblf{i}_{hph}") for hph in range(2)]
                      for i in range(2)]
            tbl_sp = dram.tile([NTP, 128], bf16, name="tblsp")
            partial_sp = dram.tile([cfg.S, H + 1], f32, name="partialsp")
            rs_out = dram.tile([SSH, H + 1], f32, name="rsout")
            st_in = [dram.tile([H, 2], f32, name=f"sti{i}") for i in range(4)]
            st_out = [dram.tile([H, 2], f32, addr_space="Shared",
                                name=f"sto{i}") for i in range(4)]

            NCHUNK = (NSH + 511) // 512

            def tsz(t):
                return min(128, NSH - t * 128)

            def csz(ci):
                return min(512, NSH - ci * 512)

            def graph_norm_relu(dst_ap, u_ap, width, n_total, stats_idx,
                                pre_b_col, gn_cols, parts=None):
                """dst = relu(S*u + B) with GN stats over u[:, :width].

                u is the pre-GN input WITHOUT the preceding linear bias
                (pre_b_col, a pf column or None); stats/affine fold it in.
                parts = (s1_parts, s2_parts) tiles of per-tile partial
                sums/square-sums already accumulated during the scatter.
                """
                s1 = vec[:, 0:1]
                if parts is not None:
                    s1p, s2p_ = parts
                    nc.vector.tensor_reduce(s1, s1p[:],
                                            mybir.AxisListType.X,
                                            mybir.AluOpType.add)
                    nc.vector.tensor_reduce(vec[:, 1:2], s2p_[:],
                                            mybir.AxisListType.X,
                                            mybir.AluOpType.add)
                else:
                    nc.vector.tensor_reduce(s1, u_ap[:, :width],
                                            mybir.AxisListType.X,
                                            mybir.AluOpType.add)
                    nch = (width + 511) // 512
                    s2p = res.tile([H, nch], f32, name=f"s2p{stats_idx}")
                    for ci in range(nch):
                        w_ = min(512, width - ci * 512)
                        nc.scalar.activation(
                            sq[:, :w_], u_ap[:, ci * 512:ci * 512 + w_],
                            mybir.ActivationFunctionType.Square,
                            accum_out=s2p[:, ci:ci + 1])
                    nc.vector.tensor_reduce(vec[:, 1:2], s2p[:],
                                            mybir.AxisListType.X,
                                            mybir.AluOpType.add)
                stv = sst.tile([H, 2], f32, name=f"stv{stats_idx}")
                nc.vector.tensor_copy(stv[:], vec[:, 0:2])
                nc.sync.dma_start(st_in[stats_idx][:], stv[:])
                nc.gpsimd.collective_compute(
                    "AllReduce", mybir.AluOpType.add,
                    replica_groups=[list(range(N_CORES))],
                    ins=[st_in[stats_idx][:].opt()],
                    outs=[st_out[stats_idx][:].opt()])
                stt = sst.tile([H, 2], f32, name=f"stt{stats_idx}")
                nc.sync.dma_start(stt[:], st_out[stats_idx][:])
                gw = pft[:, gn_cols[0]:gn_cols[0] + 1]
                gb = pft[:, gn_cols[1]:gn_cols[1] + 1]
                ga = pft[:, gn_cols[2]:gn_cols[2] + 1]
                mean = vec[:, 2:3]
                ex2 = vec[:, 3:4]
                inv_n = 1.0 / float(n_total)
                nc.vector.tensor_scalar(mean, stt[:, 0:1], inv_n, None,
                                        mybir.AluOpType.mult)
                nc.vector.tensor_scalar(ex2, stt[:, 1:2], inv_n, None,
                                        mybir.AluOpType.mult)
                if pre_b_col is not None:
                    c_ = pft[:, pre_b_col:pre_b_col + 1]
                    # mean_x = mean + c ; ex2_x = ex2 + 2*c*mean + c^2
                    t0 = vec[:, 4:5]
                    nc.vector.tensor_tensor(t0, c_, mean, mybir.AluOpType.mult)
                    nc.vector.tensor_scalar(t0, t0, 2.0, None,
                                            mybir.AluOpType.mult)
                    nc.vector.tensor_tensor(ex2, ex2, t0, mybir.AluOpType.add)
                    t1 = vec[:, 5:6]
                    nc.vector.tensor_tensor(t1, c_, c_, mybir.AluOpType.mult)
                    nc.vector.tensor_tensor(ex2, ex2, t1, mybir.AluOpType.add)
                    nc.vector.tensor_tensor(mean, mean, c_, mybir.AluOpType.add)
                # var = ex2 - mean^2 * a * (2 - a)
                m2 = vec[:, 4:5]
                nc.vector.tensor_tensor(m2, mean, mean, mybir.AluOpType.mult)
                a2 = vec[:, 5:6]
                nc.vector.tensor_scalar(a2, ga, -1.0, 2.0,
                                        mybir.AluOpType.mult,
                                        mybir.AluOpType.add)  # 2 - a
                nc.vector.tensor_tensor(a2, a2, ga, mybir.AluOpType.mult)
                nc.vector.tensor_tensor(m2, m2, a2, mybir.AluOpType.mult)
                var = vec[:, 6:7]
                nc.vector.tensor_tensor(var, ex2, m2,
                                        mybir.AluOpType.subtract)
                nc.vector.tensor_scalar(var, var, float(cfg.EPS), None,
                                        mybir.AluOpType.add)
                nc.scalar.activation(var, var,
                                     mybir.ActivationFunctionType.Sqrt)
                nc.vector.reciprocal(var, var)               # rs
                Sg = vec[:, 4:5]
                nc.vector.tensor_tensor(Sg, gw, var, mybir.AluOpType.mult)
                Bg = vec[:, 5:6]
                nc.vector.tensor_tensor(Bg, Sg, ga, mybir.AluOpType.mult)
                nc.vector.tensor_tensor(Bg, Bg, mean, mybir.AluOpType.mult)
                nc.vector.tensor_tensor(Bg, gb, Bg, mybir.AluOpType.subtract)
                if pre_b_col is not None:
                    c_ = pft[:, pre_b_col:pre_b_col + 1]
                    t0 = vec[:, 6:7]
                    nc.vector.tensor_tensor(t0, Sg, c_, mybir.AluOpType.mult)
                    nc.vector.tensor_tensor(Bg, Bg, t0, mybir.AluOpType.add)
                nc.scalar.activation(dst_ap, u_ap,
                                     mybir.ActivationFunctionType.Relu,
                                     bias=Bg, scale=Sg)

            # ================= proj layer =================
            nc.sync.dma_start(h[:cfg.IN, :NSH // 2], xT[:, :NSH // 2])
            nc.sync.dma_start(h[:cfg.IN, NSH // 2:], xT[:, NSH // 2:])
            for ci in range(NCHUNK):
                w_ = csz(ci)
                ps = mmp.tile([H, 512], f32, name=f"pj{ci}", tag="mm")
                nc.tensor.matmul(ps[:, :w_], wproj[:],
                                 h[:cfg.IN, ci * 512:ci * 512 + w_],
                                 start=True, stop=True)
                nc.vector.tensor_copy(agg[:, ci * 512:ci * 512 + w_],
                                      ps[:, :w_])
            # pf cols: 0=proj_b, (1,2,3)=gn0, 4=gcn1_b, (5,6,7)=gn1,
            #          8=gcn2_b, (9,10,11)=gn2, 12=mlp_b1, (13,14,15)=mlpgn
            graph_norm_relu(h[:H, :], agg[:], NSH, cfg.N, 0, 0, (1, 2, 3))
            if DEBUG:
                nc.sync.dma_start(dbg_h0[:], h[:H, :])

            # ================= GCN layers =================
            qn = 0
            for li, (Wt, b_col, gn_cols) in enumerate(
                    ((w1, 4, (5, 6, 7)), (w2, 8, (9, 10, 11)))):
                # node-major bf16 table t' = dinv * (h @ W), staged in two
                # separate tiles so piece-0's DMA + AllGather overlap the
                # build of piece-1 tiles (tile-granular dependency tracking)
                sg0 = stg.tile([128, SPLT, 128], bf16, name=f"sg0_{li}",
                               tag="ts0")
                sg1 = stg.tile([128, NT - SPLT, 128], bf16, name=f"sg1_{li}",
                               tag="ts1")
                nc.vector.memset(sg0[:], 0.0)
                nc.vector.memset(sg1[:], 0.0)
                for t in range(NT):
                    n_ = tsz(t)
                    ps = mmp.tile([128, H], f32, name=f"tb{li}_{t}", tag="mm")
                    nc.tensor.matmul(ps[:n_, :], h[:H, t * 128:t * 128 + n_],
                                     Wt[:], start=True, stop=True)
                    sgd = sg0[:n_, t, :H] if t < SPLT else \
                        sg1[:n_, t - SPLT, :H]
                    nc.vector.tensor_scalar(sgd, ps[:n_, :],
                                            dinv_pp[:n_, t:t + 1], None,
                                            mybir.AluOpType.mult)
                nc.sync.dma_start(
                    tbl_own[li][:SPL].rearrange("(t p) e -> p t e", p=128),
                    sg0[:])
                nc.sync.dma_start(
                    tbl_own[li][SPL:].rearrange("(t p) e -> p t e", p=128),
                    sg1[:])
                for hph in (0, 1):
                    iv = tbl_own[li][:SPL, :] if hph == 0 else \
                        tbl_own[li][SPL:, :]
                    nc.gpsimd.collective_compute(
                        "AllGather", mybir.AluOpType.bypass,
                        replica_groups=[list(range(N_CORES))],
                        ins=[iv.opt()], outs=[tbl_ch[li][hph][:].opt()])
                # self-loop init: agg = dinv * (h @ W)
                for ci in range(NCHUNK):
                    w_ = csz(ci)
                    ps = mmp.tile([H, 512], f32, name=f"sf{li}_{ci}", tag="mm")
                    nc.tensor.matmul(ps[:, :w_], Wt[:],
                                     h[:H, ci * 512:ci * 512 + w_],
                                     start=True, stop=True)
                    nc.vector.tensor_tensor(
                        agg[:, ci * 512:ci * 512 + w_], ps[:, :w_],
                        dinv_bc[:, ci * 512:ci * 512 + w_],
                        mybir.AluOpType.mult)
                # gather + scatter, two phases (AG chunks)
                s1p = res.tile([H, NT], f32, name=f"s1p{li}")
                s2p = res.tile([H, NT], f32, name=f"s2p_{li}")
                for hph in (0, 1):
                    tview = tbl_ch[li][hph][:]
                    col0 = 0 if hph == 0 else slots[0] // 16
                    blk0 = 0 if hph == 0 else nblk[0]
                    tile_of = np.repeat(np.arange(NT), S[:, hph])
                    open_ps = None
                    open_t = -1
                    for k, (o, n) in enumerate(gcalls[hph]):
                        nb = n // 128
                        g = gat.tile([128, cfg.CALL // 128, 128], bf16,
                                     name=f"g{li}_{hph}_{k}", tag="gat")
                        nc.gpsimd.dma_gather(
                            g[:, :nb, :], tview,
                            idxg[:, col0 + o // 16: col0 + (o + n) // 16],
                            n, n, 128, single_packet=False, queue_num=qn)
                        qn = (qn + 1) % 4
                        oh = ohp.tile([128, cfg.CALL // 128, 128], bf16,
                                      name=f"oh{li}_{hph}_{k}", tag="oh")
                        dlsl = dlg[:, blk0 + o // 128: blk0 + (o + n) // 128]
                        nc.vector.tensor_tensor(
                            oh[:, :nb, :],
                            iota[:].unsqueeze(1).broadcast_to([128, nb, 128]),
                            dlsl.unsqueeze(2).broadcast_to([128, nb, 128]),
                            mybir.AluOpType.is_equal)
                        for j in range(nb):
                            b = o // 128 + j
                            t = int(tile_of[b])
                            if t != open_t:
                                open_ps = scp.tile([H, 128], f32,
                                                   name=f"sc{li}_{hph}_{b}",
                                                   tag="sc")
                                open_t = t
                                first = True
                            else:
                                first = False
                            last = (b + 1 == len(tile_of)) or \
                                   (tile_of[b + 1] != t)
                            nc.tensor.matmul(open_ps[:], g[:, j, :H],
                                             oh[:, j, :],
                                             start=first, stop=last)
                            if last:
                                n_ = tsz(t)
                                av = agg[:, t * 128:t * 128 + n_]
                                nc.vector.tensor_tensor(
                                    av, av, open_ps[:, :n_],
                                    mybir.AluOpType.add)
                                if hph == 1:
                                    # finalize tile: u = agg*dinv, partial
                                    # GN stats while the scatter continues
                                    nc.vector.tensor_tensor(
                                        av, av,
                                        dinv_bc[:, t * 128:t * 128 + n_],
                                        mybir.AluOpType.mult)
                                    nc.vector.tensor_reduce(
                                        s1p[:, t:t + 1], av,
                                        mybir.AxisListType.X,
                                        mybir.AluOpType.add)
                                    nc.scalar.activation(
                                        sq[:, :n_], av,
                                        mybir.ActivationFunctionType.Square,
                                        accum_out=s2p[:, t:t + 1])
                graph_norm_relu(h[:H, :], agg[:], NSH, cfg.N,
                                1 + li, b_col, gn_cols, parts=(s1p, s2p))
                if DEBUG:
                    nc.sync.dma_start((dbg_h1 if li == 0 else dbg_h2)[:],
                                      h[:H, :])

            # ================= attention scores =================
            # u_att = relu(attn_W1.T @ h + b1); score = attn_W2.T @ u.
            # attn_b2 is dropped: softmax is shift-invariant, exp(b2)
            # cancels between numerator and denominator exactly.
            # e = exp(score) is laid out node-major [128, NT] so the spot
            # table build only depends on h (done) + this chunk's scores.
            for ci in range(NCHUNK):
                w_ = csz(ci)
                ps = mmp.tile([AH, 512], f32, name=f"at{ci}", tag="mm")
                nc.tensor.matmul(ps[:, :w_], wa1[:],
                                 h[:H, ci * 512:ci * 512 + w_],
                                 start=True, stop=True)
                uc = sst.tile([AH, 512], f32, name=f"uat{ci}", tag="uat")
                nc.scalar.activation(uc[:, :w_], ps[:, :w_],
                                     mybir.ActivationFunctionType.Relu,
                                     bias=ab1[:])
                ps2 = mmp.tile([1, 512], f32, name=f"sc2{ci}", tag="mm")
                nc.tensor.matmul(ps2[:, :w_], wa2[:], uc[:, :w_],
                                 start=True, stop=True)
                scb = sst.tile([1, 512], f32, name=f"scb{ci}", tag="uat")
                nc.vector.tensor_copy(scb[:, :w_], ps2[:, :w_])
                nbj = (w_ + 127) // 128
                for j in range(nbj):
                    wj = min(128, w_ - j * 128)
                    pt = mmp.tile([128, 1], f32, name=f"et{ci}_{j}",
                                  tag="tpose")
                    nc.tensor.transpose(pt[:wj, :1],
                                        scb[:1, j * 128:j * 128 + wj],
                                        ident[:1, :1])
                    nc.scalar.activation(
                        e_pp[:wj, 4 * ci + j:4 * ci + j + 1], pt[:wj, :1],
                        mybir.ActivationFunctionType.Exp)

            # ======== spot table rows: [e*h (H) | e | pad] bf16 ========
            sp0 = stg.tile([128, SPLT, 128], bf16, name="sgsp0", tag="ts0")
            sp1 = stg.tile([128, NT - SPLT, 128], bf16, name="sgsp1",
                           tag="ts1")
            nc.vector.memset(sp0[:], 0.0)
            nc.vector.memset(sp1[:], 0.0)
            for t in range(NT):
                n_ = tsz(t)
                ps = mmp.tile([128, H], f32, name=f"tr{t}", tag="tpose")
                nc.tensor.transpose(ps[:n_, :H],
                                    h[:H, t * 128:t * 128 + n_],
                                    ident[:H, :H])
                spd = sp0 if t < SPLT else sp1
                td = t if t < SPLT else t - SPLT
                nc.vector.tensor_scalar(spd[:n_, td, :H], ps[:n_, :H],
                                        e_pp[:n_, t:t + 1], None,
                                        mybir.AluOpType.mult)
                nc.vector.tensor_copy(spd[:n_, td, H:H + 1],
                                      e_pp[:n_, t:t + 1])
            nc.sync.dma_start(
                tbl_sp[:SPL].rearrange("(t p) e -> p t e", p=128), sp0[:])
            nc.sync.dma_start(
                tbl_sp[SPL:].rearrange("(t p) e -> p t e", p=128), sp1[:])

            # ===== local spot scatter: partial sums per 128-spot group =====
            nc.vector.memset(stage_sp[:], 0.0)
            for sph in (0, 1):
                tview = tbl_sp[:cfg.SPL] if sph == 0 else tbl_sp[cfg.SPL:]
                col0 = 0 if sph == 0 else sslots[0] // 16
                blk0 = 0 if sph == 0 else nblk_sp[0]
                tile_of_sp = np.repeat(np.arange(GT), NBG[:, sph])
                open_ps = None
                open_g = -1
                for k, (o, n) in enumerate(scalls[sph]):
                    nb = n // 128
                    g = gat.tile([128, cfg.CALL // 128, 128], bf16,
                                 name=f"gs{sph}_{k}", tag="gat")
                    nc.gpsimd.dma_gather(
                        g[:, :nb, :], tview,
                        idxs_sp[:, col0 + o // 16:col0 + (o + n) // 16],
                        n, n, 128, single_packet=False, queue_num=qn)
                    qn = (qn + 1) % 4
                    oh = ohp.tile([128, cfg.CALL // 128, 128], bf16,
                                  name=f"ohs{sph}_{k}", tag="oh")
                    plsl = plsp[:, blk0 + o // 128:blk0 + (o + n) // 128]
                    nc.vector.tensor_tensor(
                        oh[:, :nb, :],
                        iota[:].unsqueeze(1).broadcast_to([128, nb, 128]),
                        plsl.unsqueeze(2).broadcast_to([128, nb, 128]),
                        mybir.AluOpType.is_equal)
                    for j in range(nb):
                        b = o // 128 + j
                        gi = int(tile_of_sp[b])
                        if gi != open_g:
                            open_ps = scp.tile([128, H + 1], f32,
                                               name=f"scs{sph}_{b}", tag="sc")
                            open_g = gi
                            first = True
                        else:
                            first = False
                        last = (b + 1 == len(tile_of_sp)) or \
                               (tile_of_sp[b + 1] != gi)
                        nc.tensor.matmul(open_ps[:], oh[:, j, :],
                                         g[:, j, :H + 1],
                                         start=first, stop=last)
                        if last:
                            nc.vector.tensor_tensor(
                                stage_sp[:, gi, :], stage_sp[:, gi, :],
                                open_ps[:], mybir.AluOpType.add)
            gfull = (cfg.S // 128) * 128
            nc.sync.dma_start(
                partial_sp[:gfull, :].rearrange("(g p) f -> p g f", p=128),
                stage_sp[:, :cfg.S // 128, :])
            if cfg.S % 128:
                nc.sync.dma_start(partial_sp[gfull:, :],
                                  stage_sp[:cfg.S % 128, GT - 1, :])
            if DEBUG:
                nc.sync.dma_start(dbg_ps[:], partial_sp[:])
            nc.gpsimd.collective_compute(
                "ReduceScatter", mybir.AluOpType.add,
                replica_groups=[list(range(N_CORES))],
                ins=[partial_sp[:].opt()], outs=[rs_out[:].opt()])

            # ===== normalize spot vectors, back to feature-major =====
            nc.vector.memset(spot_fm[:], 0.0)
            for gi in range(ST):
                gsz = min(128, SSH - gi * 128)
                nc.sync.dma_start(rs_sb[:gsz, gi, :],
                                  rs_out[gi * 128:gi * 128 + gsz, :])
                den = sst.tile([128, 1], f32, name=f"den{gi}", tag="den")
                nc.vector.tensor_scalar(den[:gsz], rs_sb[:gsz, gi, H:H + 1],
                                        1e-30, None, mybir.AluOpType.max)
                nc.vector.reciprocal(den[:gsz], den[:gsz])
                nc.vector.tensor_scalar(rs_sb[:gsz, gi, :H],
                                        rs_sb[:gsz, gi, :H],
                                        den[:gsz], None,
                                        mybir.AluOpType.mult)
                ps = mmp.tile([H, 128], f32, name=f"spt{gi}", tag="tpose")
                nc.tensor.transpose(ps[:, :gsz], rs_sb[:gsz, gi, :H],
                                    ident[:gsz, :gsz])
                nc.vector.tensor_copy(spot_fm[:, gi * 128:gi * 128 + gsz],
                                      ps[:, :gsz])
            if DEBUG:
                nc.sync.dma_start(dbg_sp[:], spot_fm[:])

            # ================= MLP head =================
            um = res.tile([H, ST * 128], f32, name="um")
            for ci in range((ST * 128 + 511) // 512):
                w_ = min(512, ST * 128 - ci * 512)
                ps = mmp.tile([H, 512], f32, name=f"m1{ci}", tag="mm")
                nc.tensor.matmul(ps[:, :w_], wm1[:],
                                 spot_fm[:, ci * 512:ci * 512 + w_],
                                 start=True, stop=True)
                nc.vector.tensor_copy(um[:, ci * 512:ci * 512 + w_],
                                      ps[:, :w_])
            graph_norm_relu(um[:], um[:], SSH, cfg.S, 3, 12, (13, 14, 15))
            zo = res.tile([OD, ST * 128], f32, name="zo")
            for ci in range((ST * 128 + 511) // 512):
                w_ = min(512, ST * 128 - ci * 512)
                ps = mmp.tile([OD, 512], f32, name=f"m2{ci}", tag="mm")
                nc.tensor.matmul(ps[:, :w_], wm2[:],
                                 um[:, ci * 512:ci * 512 + w_],
                                 start=True, stop=True)
                nc.vector.tensor_scalar(zo[:, ci * 512:ci * 512 + w_],
                                        ps[:, :w_], mb2[:], None,
                                        mybir.AluOpType.add)
            for gi in range(ST):
                n_ = min(128, SSH - gi * 128)
                if n_ <= 0:
                    break
                ps = mmp.tile([128, OD], f32, name=f"ot{gi}", tag="tpose")
                nc.tensor.transpose(ps[:, :], zo[:, gi * 128:(gi + 1) * 128],
                                    ident[:OD, :OD])
                sg = sst.tile([128, OD], f32, name=f"os{gi}", tag="ostg")
                nc.vector.tensor_copy(sg[:], ps[:])
                nc.sync.dma_start(out[gi * 128:gi * 128 + n_, :], sg[:n_, :])

    nc.compile()
    return nc


_CACHE = {}


def _build_inputs(cfg, inputs, idx_w, dl_w, idxs_w, pl_w, dinv):
    f = np.float32
    x = np.asarray(inputs['x'], f)
    xT = np.ascontiguousarray(x.T)

    def col(v):
        return np.asarray(v, f).reshape(-1, 1)

    pf = np.zeros((cfg.H, 16), f)
    for i, k in enumerate(['proj_b', 'gn0_w', 'gn0_b', 'gn0_a',
                           'gcn1_b', 'gn1_w', 'gn1_b', 'gn1_a',
                           'gcn2_b', 'gn2_w', 'gn2_b', 'gn2_a',
                           'mlp_b1', 'mlp_gn_w', 'mlp_gn_b', 'mlp_gn_a']):
        pf[:, i] = np.asarray(inputs[k], f)
    import ml_dtypes
    iota = np.broadcast_to(np.arange(128, dtype=f),
                           (128, 128)).astype(ml_dtypes.bfloat16)
    ident = np.eye(128, dtype=f)
    in_maps = []
    for c in range(N_CORES):
        n0 = c * cfg.NSH
        dinv_own = dinv[n0:n0 + cfg.NSH]
        dpp = np.ones((128, cfg.NT), f)
        for t in range(cfg.NT):
            n_ = min(128, cfg.NSH - t * 128)
            dpp[:n_, t] = dinv_own[t * 128:t * 128 + n_]
        dbc = np.broadcast_to(dinv_own[None, :], (cfg.H, cfg.NSH)).copy()
        in_maps.append({
            'xT': np.ascontiguousarray(xT[:, n0:n0 + cfg.NSH]),
            'dinv_pp': dpp, 'dinv_bc': dbc,
            'iota_in': iota, 'ident_in': ident,
            'idx_gcn': idx_w[c],
            'dl_gcn': dl_w[c].astype(ml_dtypes.bfloat16),
            'idx_spot': idxs_w[c],
            'pl_spot': pl_w[c].astype(ml_dtypes.bfloat16),
            'projW': np.asarray(inputs['proj_W'], f),
            'W1': np.asarray(inputs['gcn1_W'], f),
            'W2': np.asarray(inputs['gcn2_W'], f),
            'attnW1': np.asarray(inputs['attn_W1'], f),
            'attnW2': np.asarray(inputs['attn_W2'], f),
            'mlpW1': np.asarray(inputs['mlp_W1'], f),
            'mlpW2': np.asarray(inputs['mlp_W2'], f),
            'pf': pf,
            'attn_b1': col(inputs['attn_b1']),
            'attn_b2': col(inputs['attn_b2']),
            'mlp_b2': col(inputs['mlp_b2']),
        })
    return in_maps


def kernel(**inputs):
    from concourse import bass_utils
    cfg = Cfg(n_nodes=int(np.asarray(inputs['x']).shape[0]),
              n_edges=int(np.asarray(inputs['edge_index']).shape[1]),
              in_dim=int(np.asarray(inputs['x']).shape[1]),
              hid=int(np.asarray(inputs['proj_W']).shape[1]),
              attn_hid=int(np.asarray(inputs['attn_W1']).shape[1]),
              out_dim=int(np.asarray(inputs['mlp_W2']).shape[1]),
              n_spots=int(inputs['num_spots']))
    ei = np.asarray(inputs['edge_index']).astype(np.int64)
    cts = np.asarray(inputs['cell_to_spot']).astype(np.int64)
    src, dst = ei[0], ei[1]
    deg = (np.bincount(dst, minlength=cfg.N) + 1).astype(np.float64)
    dinv = (1.0 / np.sqrt(deg)).astype(np.float32)

    idx_w, dl_w, emeta = _prep_edges(cfg, src, dst)
    idxs_w, pl_w, smeta = _prep_spots(cfg, cts)

    key = (cfg.N, cfg.E, tuple(emeta['nblk']), tuple(smeta['nblk_sp']))
    if key not in _CACHE:
        _CACHE[key] = build_program(cfg, emeta, smeta)
    nc = _CACHE[key]

    in_maps = _build_inputs(cfg, inputs, idx_w, dl_w, idxs_w, pl_w, dinv)
    res = bass_utils.run_bass_kernel_spmd(
        nc, in_maps, core_ids=list(range(N_CORES)),
        trace=os.environ.get('KERNEL_TRACE', '0') == '1',
        tmpdir=os.environ.get('KERNEL_TMPD'))
    if os.environ.get('KERNEL_TRACE', '0') == '1':
        print('HW exec time:', res.exec_time_ns, 'ns')
    out = np.concatenate([res.results[c]['out'] for c in range(N_CORES)],
                         axis=0)
    return out.astype(np.float32)
